# revision 19
# baseline (speedup 1.0000x reference)
"""Trainium2 Bass kernel for EnhancedLocalComplexAttention.

Reference semantics (complex windowed attention):
  x = x_re + i*x_im                     [b=2, n=4096, dim=512]
  q = x @ wq.T ; k = x @ wk.T ; v = x @ wv.T          (complex, 512x512)
  per head (8 heads x 64) and non-overlapping 128-token window:
    dots = real(q . conj(k)) * scale + rel_bias[j-i+128]
    attn = softmax(dots); out = attn @ v  (attn real)
  y = out @ wo.T  (complex); return stack([y.re, y.im])  [2, b, n, dim]

Sharding: data-parallel over tokens. Core c gets tokens [c*512,(c+1)*512)
of each batch (windows are 128-aligned, so fully local).

Optimizations over the straightforward version:
  - Gauss 3-multiplication complex matmul for all four projections:
    K1 = xsum @ c, K2 = x_re @ (d-c), K3 = x_im @ (c+d)  (c/d = Wre.T/Wim.T)
    re = K1-K3, im = K1+K2. Host builds the weight combinations and xsum;
    25% fewer PE cycles, and the combines replace the PSUM->SBUF copies the
    4-mult version needed anyway. K1 is copied to SBUF right after it lands
    (engines may read only ONE PSUM operand per instruction, and this frees
    K1's PSUM bank early enough that all K tags are single-buffered).
  - Attention per head (4 windows batched in one PSUM bank): softmax
    (dots+bias) computed as exp(dots)*exp(bias) with host-precomputed
    exp(bias), one exp over [128,512] with no accumulator, segmented
    row-sum on DVE, and the attn transpose+normalize fused into one PE
    matmul against diag(1/s) built per window with tensor_scalar_mul.
  - bf16 on all matmul inputs; fp32 PSUM accumulation everywhere.
  - Batch 1's projections interleave with batch 0's attention (and batch 0's
    output projection with batch 1's attention) so the PE never drains.

HW constraints learned the hard way (neuronxcc verifier / device aborts):
  - tensor ops may read at most one PSUM operand; GPSIMD may not touch PSUM
    at all; PE matmul operands must start at SBUF partition 0 (base-64
    operands hit the broken quadrant-3 XBUS path); tensor_tensor_reduce and
    gpsimd tensor ops abort the execution unit on this runtime.
"""

import numpy as np
import ml_dtypes

P = 128         # SBUF partitions / window size
DIM = 512
NKT = DIM // P  # 4 k-tiles
TOK = 512       # tokens per core per batch
NIT = TOK // P  # 4 token tiles (= windows) per chunk
NB = 2          # batches
NH = 8          # heads
HD = 64         # head dim
N_CORES = 8
N = 4096
SCALE = HD ** (-0.5)
NU = NH          # attention units per batch (one per head, 4 windows each)

# w{p}1 = Wp_re.T (q: *SCALE); w{p}2 = Wp_im.T - Wp_re.T; w{p}3 = Wp_re.T + Wp_im.T
W_NAMES = [f"w{p}{i}" for p in "qkvo" for i in (1, 2, 3)]

_COMPILED = {}
LAST_RESULT = None


def _build_program(loop_n=None, phases=("attn", "oproj")):
    import concourse.bacc as bacc
    import concourse.mybir as mybir
    import concourse.tile as tile
    from contextlib import ExitStack

    f32 = mybir.dt.float32
    bf16 = mybir.dt.bfloat16
    Exp = mybir.ActivationFunctionType.Exp
    ADD = mybir.AluOpType.add
    AX = mybir.AxisListType.X

    nc = bacc.Bacc(
        "TRN2",
        target_bir_lowering=False,
        debug=False,
        enable_asserts=False,
        num_devices=N_CORES,
    )

    ins = {}
    for name in ["xT_re", "xT_im", "xT_sum"]:
        ins[name] = nc.dram_tensor(name, [NB, DIM, TOK], bf16, kind="ExternalInput").ap()
    for name in W_NAMES:
        ins[name] = nc.dram_tensor(name, [DIM, DIM], bf16, kind="ExternalInput").ap()
    ins["eb4"] = nc.dram_tensor("eb4", [P, NIT * P], f32, kind="ExternalInput").ap()
    ins["ident"] = nc.dram_tensor("ident", [P, P], bf16, kind="ExternalInput").ap()
    outs = {
        "y_re": nc.dram_tensor("y_re", [NB, TOK, DIM], f32, kind="ExternalOutput").ap(),
        "y_im": nc.dram_tensor("y_im", [NB, TOK, DIM], f32, kind="ExternalOutput").ap(),
    }

    with tile.TileContext(nc) as tc, ExitStack() as ctx:
        wpool = ctx.enter_context(tc.tile_pool(name="wpool", bufs=1))
        cpool = ctx.enter_context(tc.tile_pool(name="cpool", bufs=1))
        xpool = ctx.enter_context(tc.tile_pool(name="xpool", bufs=2))
        qkpool = ctx.enter_context(tc.tile_pool(name="qkpool", bufs=2))
        vpool = ctx.enter_context(tc.tile_pool(name="vpool", bufs=2))
        aopool = ctx.enter_context(tc.tile_pool(name="aopool", bufs=2))
        ypool = ctx.enter_context(tc.tile_pool(name="ypool", bufs=4))
        sc = ctx.enter_context(tc.tile_pool(name="sc", bufs=6))
        pk = ctx.enter_context(tc.tile_pool(name="pk", bufs=1, space="PSUM"))
        pdp = ctx.enter_context(tc.tile_pool(name="pdp", bufs=3, space="PSUM"))
        pap = ctx.enter_context(tc.tile_pool(name="pap", bufs=2, space="PSUM"))

        # --- resident constants -------------------------------------------
        # All weight DMAs on the gpsimd queue (cheapest issue), ordered by
        # first use (K1 group runs first within each projection).
        w_sb = {}
        for pfx in "qkvo":
            for i in (1, 2, 3):
                name = f"w{pfx}{i}"
                t = wpool.tile([P, NKT, DIM], bf16, name=f"sb_{name}")
                nc.gpsimd.dma_start(out=t, in_=ins[name].rearrange("(kt p) j -> p kt j", p=P))
                w_sb[name] = t
        eb4_sb = cpool.tile([P, NIT, P], f32, name="eb4_sb")
        nc.sync.dma_start(out=eb4_sb, in_=ins["eb4"].rearrange("p (w t) -> p w t", w=NIT))
        id_sb = cpool.tile([P, P], bf16, name="id_sb")
        nc.sync.dma_start(out=id_sb, in_=ins["ident"])

        def body():
          # ---- x DMAs (sync queue); batch 0 split per-kt for fast start ---
          x_sb = []
          for b in range(NB):
            xre = xpool.tile([P, NKT, TOK], bf16, tag="xre", name=f"xre{b}")
            xim = xpool.tile([P, NKT, TOK], bf16, tag="xim", name=f"xim{b}")
            xsum = xpool.tile([P, NKT, TOK], bf16, tag="xsum", name=f"xsum{b}")
            for t, src in ((xsum, "xT_sum"), (xre, "xT_re"), (xim, "xT_im")):
                if b == 0:
                    for kt in range(NKT):
                        nc.sync.dma_start(
                            out=t[:, kt, :],
                            in_=ins[src][b, kt * P:(kt + 1) * P, :])
                else:
                    nc.sync.dma_start(
                        out=t, in_=ins[src][b].rearrange("(kt p) t -> p kt t", p=P))
            x_sb.append((xre, xim, xsum))

          qk_sb, v_sb, ao_sb = [], [], []
          for b in range(NB):
            qk_sb.append((
                # cat layout: partitions 0:64 = re(d), 64:128 = im(d), per head
                qkpool.tile([P, NH, TOK], bf16, tag="qcat", name=f"qcat{b}"),
                qkpool.tile([P, NH, TOK], bf16, tag="kcat", name=f"kcat{b}"),
            ))
            v_sb.append(vpool.tile([P, NIT, NH, 2, HD], bf16, tag="vcat", name=f"vcat{b}"))
            ao_sb.append((
                aopool.tile([P, NKT, TOK], bf16, tag="aore", name=f"aore{b}"),
                aopool.tile([P, NKT, TOK], bf16, tag="aoim", name=f"aoim{b}"),
                aopool.tile([P, NKT, TOK], bf16, tag="aosum", name=f"aosum{b}"),
            ))

          # ---- building blocks -------------------------------------------
          # PSUM group order K1,K2,K3. Engines may read only ONE PSUM operand
          # per instruction, so K1 is copied to SBUF right after it lands
          # (also frees its bank early -> all K tags single-buffered); each
          # combine then reads k1sb + one PSUM operand.
          def gauss_mms(gname, specs, k1_dt, k1_eng):
            ks = {}
            for tag, w, xs, lhs_is_w, msl in specs:
                ps = pk.tile([P, TOK], f32, tag=tag, name=f"{tag}_{gname}")
                for kt in range(NKT):
                    if lhs_is_w:
                        nc.tensor.matmul(ps, w[:, kt, msl], xs[:, kt, :],
                                         start=(kt == 0), stop=(kt == NKT - 1))
                    else:
                        nc.tensor.matmul(ps, xs[:, kt, msl], w[:, kt, :],
                                         start=(kt == 0), stop=(kt == NKT - 1))
                ks[tag] = ps
            k1sb = sc.tile([P, TOK], k1_dt, tag=f"k1sb_{k1_dt}",
                           name=f"k1sb_{gname}", bufs=3)
            k1_eng(out=k1sb, in_=ks["K1"])
            return k1sb, ks["K2"], ks["K3"]

          def qk_proj_group(b, di, jt):
            """dst[di]: 0=q, 1=k. Produces feature tile jt of qcat/kcat."""
            dst = qk_sb[b][di]
            pfx = "qk"[di]
            xre, xim, xsum = x_sb[b]
            jsl = slice(jt * P, (jt + 1) * P)
            k1, k2, k3 = gauss_mms(f"{pfx}{b}{jt}", (
                ("K1", w_sb[f"w{pfx}1"], xsum, True, jsl),
                ("K2", w_sb[f"w{pfx}2"], xre, True, jsl),
                ("K3", w_sb[f"w{pfx}3"], xim, True, jsl),
            ), bf16, nc.scalar.copy)
            nc.vector.tensor_sub(dst[0:HD, 2 * jt, :], k1[0:HD, :], k3[0:HD, :])
            nc.vector.tensor_sub(dst[0:HD, 2 * jt + 1, :], k1[HD:P, :], k3[HD:P, :])
            nc.vector.tensor_add(dst[HD:P, 2 * jt, :], k1[0:HD, :], k2[0:HD, :])
            nc.vector.tensor_add(dst[HD:P, 2 * jt + 1, :], k1[HD:P, :], k2[HD:P, :])

          def v_proj_group(b, it):
            vcat = v_sb[b]
            xre, xim, xsum = x_sb[b]
            isl = slice(it * P, (it + 1) * P)
            k1, k2, k3 = gauss_mms(f"v{b}{it}", (
                ("K1", w_sb["wv1"], xsum, False, isl),
                ("K2", w_sb["wv2"], xre, False, isl),
                ("K3", w_sb["wv3"], xim, False, isl),
            ), bf16, nc.scalar.copy)
            k1r = k1.rearrange("p (h d) -> p h d", h=NH)
            nc.vector.tensor_add(vcat[:, it, :, 1, :], k1r,
                                 k2.rearrange("p (h d) -> p h d", h=NH))
            nc.vector.tensor_sub(vcat[:, it, :, 0, :], k1r,
                                 k3.rearrange("p (h d) -> p h d", h=NH))

          def o_proj_group(b, it):
            aore, aoim, aosum = ao_sb[b]
            isl = slice(it * P, (it + 1) * P)
            k1, k2, k3 = gauss_mms(f"o{b}{it}", (
                ("K1", w_sb["wo1"], aosum, False, isl),
                ("K2", w_sb["wo2"], aore, False, isl),
                ("K3", w_sb["wo3"], aoim, False, isl),
            ), f32, nc.vector.tensor_copy)
            yim = ypool.tile([P, DIM], f32, tag="y", name=f"yim{b}{it}")
            nc.vector.tensor_add(yim, k1, k2)
            nc.gpsimd.dma_start(out=outs["y_im"][b, isl, :], in_=yim)
            yre = ypool.tile([P, DIM], f32, tag="y", name=f"yre{b}{it}")
            nc.vector.tensor_sub(yre, k1, k3)
            nc.gpsimd.dma_start(out=outs["y_re"][b, isl, :], in_=yre)

          # ---- attention unit u = head h; all 4 windows in one PSUM bank --
          # softmax(dots+bias) via exp(dots)*exp(bias): the bias multiply and
          # the row-sum fuse into one tensor_tensor_reduce per window, so the
          # bias never touches PSUM.
          def attn_front(b, h):
            qcat, kcat = qk_sb[b]
            pd = pdp.tile([P, NIT, P], f32, tag="pd", name=f"pd{b}_{h}")
            for w in range(NIT):
                wsl = slice(w * P, (w + 1) * P)
                nc.tensor.matmul(pd[:, w, :], qcat[:, h, wsl], kcat[:, h, wsl],
                                 start=True, stop=True)
            eraw = sc.tile([P, NIT, P], bf16, tag="eraw", name=f"eraw{b}_{h}", bufs=3)
            nc.scalar.activation(out=eraw, in_=pd, func=Exp)
            e4 = sc.tile([P, NIT, P], bf16, tag="e", name=f"e{b}_{h}", bufs=6)
            nc.vector.tensor_mul(e4, eraw, eb4_sb)
            s4 = sc.tile([P, NIT], f32, tag="s", name=f"s{b}_{h}", bufs=6)
            nc.vector.tensor_reduce(out=s4, in_=e4, axis=AX, op=ADD)
            rcp4 = sc.tile([P, NIT], f32, tag="r", name=f"r{b}_{h}", bufs=6)
            nc.vector.reciprocal(rcp4, s4)
            diag = sc.tile([P, NIT, P], bf16, tag="dg", name=f"dg{b}_{h}", bufs=6)
            for w in range(NIT):
                nc.vector.tensor_scalar_mul(diag[:, w, :], id_sb, rcp4[:, w:w + 1])
            return (e4, diag)

          def attn_back(b, h, ed):
            e4, diag = ed
            jt, off = h // 2, (h % 2) * HD
            vcat = v_sb[b]
            aore, aoim, _ = ao_sb[b]
            pt = pap.tile([P, NIT, P], f32, tag="pa", name=f"pt{b}_{h}")
            for w in range(NIT):
                # pt[:,w] = e4[:,w].T @ diag(rcp): transpose + normalize
                nc.tensor.matmul(pt[:, w, :], e4[:, w, :], diag[:, w, :],
                                 start=True, stop=True)
            at4 = sc.tile([P, NIT, P], bf16, tag="at", name=f"at{b}_{h}", bufs=3)
            nc.scalar.copy(out=at4, in_=pt)
            pv = pap.tile([P, NIT, P], f32, tag="pa", name=f"pv{b}_{h}")
            for w in range(NIT):
                nc.tensor.matmul(pv[:, w, :], vcat[:, w, h, :, :], at4[:, w, :],
                                 start=True, stop=True)
            nc.scalar.copy(out=aore[off:off + HD, jt, :],
                           in_=pv[0:HD].rearrange("p w t -> p (w t)"))
            nc.scalar.copy(out=aoim[off:off + HD, jt, :],
                           in_=pv[HD:P].rearrange("p w t -> p (w t)"))

          def ao_sum(b):
            aore, aoim, aosum = ao_sb[b]
            nc.vector.tensor_add(aosum, aore, aoim)

          # ---- phase A/B: batch-0 projections ----------------------------
          for di in range(2):
              for jt in range(NKT):
                  qk_proj_group(0, di, jt)
          for it in range(NIT):
              v_proj_group(0, it)

          # ---- phase C: b0 attention || b1 q/k projections ---------------
          LA = 3
          fronts = {}
          for i in range(NU + LA):
              if i < NU:
                  fronts[i] = attn_front(0, i)
              if i < 2 * NKT:
                  qk_proj_group(1, i // NKT, i % NKT)
              if i >= LA:
                  attn_back(0, i - LA, fronts.pop(i - LA))
          ao_sum(0)

          # ---- phase D: b1 attention || b1 v proj + b0 o proj ------------
          LA2 = 5
          fillers = {0: ("v", 0), 1: ("v", 1), 2: ("v", 2), 3: ("v", 3),
                     4: ("o", 0), 6: ("o", 1), 8: ("o", 2), 10: ("o", 3)}
          for i in range(NU + LA2):
              if i < NU:
                  fronts[i] = attn_front(1, i)
              if i in fillers:
                  kind, j = fillers[i]
                  if kind == "v":
                      v_proj_group(1, j)
                  else:
                      o_proj_group(0, j)
              if i >= LA2:
                  attn_back(1, i - LA2, fronts.pop(i - LA2))
          ao_sum(1)

          # ---- phase E: b1 output projection -----------------------------
          for it in range(NIT):
              o_proj_group(1, it)

        if loop_n:
            with tc.For_i(0, loop_n):
                body()
        else:
            body()

    nc.compile()
    return nc


def get_compiled(loop_n=None, phases=("attn", "oproj")):
    key = (loop_n, tuple(phases))
    if key not in _COMPILED:
        _COMPILED[key] = _build_program(loop_n, phases)
    return _COMPILED[key]


def make_in_maps(x_re, x_im, wq_re, wq_im, wk_re, wk_im, wv_re, wv_im,
                 wo_re, wo_im, rel_bias):
    """Host-side prep: shard x over token chunks, replicate Gauss-combined
    transposed weights (bf16)."""
    f32 = np.float32
    bf = ml_dtypes.bfloat16
    x_re = np.asarray(x_re, f32)
    x_im = np.asarray(x_im, f32)
    x_sum = x_re + x_im

    shared = {}
    for pfx, wre, wim in (("q", wq_re, wq_im), ("k", wk_re, wk_im),
                          ("v", wv_re, wv_im), ("o", wo_re, wo_im)):
        c = np.asarray(wre, f32).T
        d = np.asarray(wim, f32).T
        if pfx == "q":
            c = c * SCALE
            d = d * SCALE
        shared[f"w{pfx}1"] = np.ascontiguousarray(c).astype(bf)
        shared[f"w{pfx}2"] = np.ascontiguousarray(d - c).astype(bf)
        shared[f"w{pfx}3"] = np.ascontiguousarray(c + d).astype(bf)

    idx = np.arange(P)[None, :] - np.arange(P)[:, None] + P
    bias_mat = np.asarray(rel_bias, f32)[idx]
    shared["eb4"] = np.ascontiguousarray(
        np.concatenate([np.exp(bias_mat)] * NIT, axis=1))
    shared["ident"] = np.eye(P, dtype=bf)

    in_maps = []
    for c_ in range(N_CORES):
        sl = slice(c_ * TOK, (c_ + 1) * TOK)
        m = dict(shared)
        m["xT_re"] = np.ascontiguousarray(x_re[:, sl, :].transpose(0, 2, 1)).astype(bf)
        m["xT_im"] = np.ascontiguousarray(x_im[:, sl, :].transpose(0, 2, 1)).astype(bf)
        m["xT_sum"] = np.ascontiguousarray(x_sum[:, sl, :].transpose(0, 2, 1)).astype(bf)
        in_maps.append(m)
    return in_maps


def assemble_output(results):
    out = np.empty((2, NB, N, DIM), np.float32)
    for c in range(N_CORES):
        sl = slice(c * TOK, (c + 1) * TOK)
        out[0, :, sl, :] = results[c]["y_re"]
        out[1, :, sl, :] = results[c]["y_im"]
    return out


def kernel(**inputs):
    global LAST_RESULT
    import os
    from concourse.bass_utils import run_bass_kernel_spmd

    nc = get_compiled()
    in_maps = make_in_maps(**inputs)
    core_ids = list(range(N_CORES))
    try:
        res = run_bass_kernel_spmd(nc, in_maps, core_ids)
    except ModuleNotFoundError:
        # BASS_TRACE set but this container lacks the axon NTFF hook module;
        # rerun with tracing hard-disabled.
        os.environ["BASS_NEVER_TRACE"] = "1"
        res = run_bass_kernel_spmd(nc, in_maps, core_ids)
    LAST_RESULT = res
    return assemble_output(res.results)
